# revision 15
# baseline (speedup 1.0000x reference)
"""Trainium2 Bass kernel for nn_ATC_Network (2-layer GCN + BN + LeakyReLU).

Computation (see reference):
    row, col, w  (+ self loops w=1)
    deg[c] = sum_{e: col=c} w_e ;  dis = rsqrt(deg)
    norm_e = dis[row]*w*dis[col]
    z1[c]  = sum_e norm_e * x[row]          (conv1 aggregate, incl self loop)
    y1     = z1 @ W1 ; x2 = LeakyReLU(BN(y1))        (conv bias cancels in BN)
    z2[c]  = sum_e norm_e * (x2 @ W2)[row]
    out    = BN(z2)

v3 design notes:
  - dst-sharded across 8 cores (NP = 6250 dst nodes per core); graph norm
    host-side; self-loops are ordinary edges.
  - ONE padded slot-stream per core drives BOTH conv layers (shared
    weighted one-hot table ws).  Cells are (src-chunk h, dst-window w):
    13 windows of 512 dsts, 2 src chunks.  Slots within a 128-block are
    sorted by source index for HBM locality.
  - conv1 gathers 256B x16 rows from a chunk-permuted x16_arr (chunk-local
    indices fit int16).  PSUM [128,512] per window; chunk-A pass closes
    with a copy, chunk-B pass closes with an add.
  - conv2 pre-folds W2: u2 = x2@W2 (64 wide), AllGathered f16 in TWO
    chunks (A = local rows [0:3072] = windows 0-5) so the second AG
    overlaps conv2's chunk-A gathers.  Gathers fetch 256B pairs of u2
    rows; a DVE parity select picks the right 64 columns.
  - BN stats: DVE free-dim reduces over the feature-major activations;
    AllReduce payload is [128,2] f32.
"""

import sys

sys.path.insert(0, "/opt/trn_rl_repo")

import numpy as np
import ml_dtypes

import concourse.bass as bass
import concourse.tile as tile
from concourse import bacc, bass_utils, mybir

FP32 = mybir.dt.float32
F16 = mybir.dt.float16
I16 = mybir.dt.int16

# ---------------------------------------------------------------- config ----
CFG = dict(
    N=50000, E=800000, F=128, H=128, O=64, NCORE=8,
    WIN=512,             # dsts per PSUM window
    CA=3072,             # chunk-A rows per core (= 6 windows)
    GCH=512,             # slots per dma_gather call (SWDGE ring: 1024 descs)
    EPS=1e-5, NEG=0.01,
)


# ---------------------------------------------------------- preprocessing ---
def preprocess(adj, w, cfg):
    """Host-side graph prep: norm, self-loops, per-core padded slot streams.

    Uniform block structure across cores (SPMD: one instruction stream;
    per-core data differs).
    """
    N, E, NCORE = cfg["N"], cfg["E"], cfg["NCORE"]
    WIN, CA = cfg["WIN"], cfg["CA"]
    NP = N // NCORE
    CB = NP - CA
    NA, NB_ = NCORE * CA, NCORE * CB        # chunk region sizes (24576, 25424)
    W = (NP + WIN - 1) // WIN               # 13 windows
    NCELL = 2 * W

    row0 = np.asarray(adj[0], np.int64)
    col0 = np.asarray(adj[1], np.int64)
    w = np.asarray(w, np.float32)

    # gcn_norm host-side (incl self loops, weight 1)
    deg = np.zeros(N, np.float64)
    np.add.at(deg, col0, w.astype(np.float64))
    deg += 1.0
    dis = (1.0 / np.sqrt(deg)).astype(np.float32)

    row = np.concatenate([row0, np.arange(N, dtype=np.int64)])
    col = np.concatenate([col0, np.arange(N, dtype=np.int64)])
    nrm = np.concatenate([dis[row0] * w * dis[col0], dis * dis]).astype(np.float32)
    EA = E + N

    # chunk-permuted source positions
    g = np.arange(N, dtype=np.int64)
    core_g, loc_g = g // NP, g % NP
    inA = loc_g < CA
    posA = core_g * CA + loc_g
    posB = core_g * CB + (loc_g - CA)
    poschunk = np.where(inA, posA, posB)    # position within own chunk region
    posfull = np.where(inA, posA, NA + posB)  # row in x16_arr

    # per-edge attributes
    core_e = col // NP
    lc = col % NP
    w_e = lc // WIN
    dl = lc - w_e * WIN
    h_e = (~inA[row]).astype(np.int64)      # src chunk
    ip = poschunk[row]                      # chunk-local gather index (conv1)

    # stable sort by (core, h, w, dl)
    key = ((core_e * 2 + h_e) * W + w_e) * WIN + dl
    order = np.argsort(key, kind="stable")

    # caps per cell (h, w): max over cores, ceil to 128
    chw = (core_e * 2 + h_e) * W + w_e
    cnt = np.bincount(chw, minlength=NCORE * NCELL).reshape(NCORE, NCELL)
    cap = np.maximum(cnt.max(0), 1)
    cap = ((cap + 127) // 128) * 128        # [NCELL]
    tsb = np.zeros(NCELL + 1, np.int64)
    tsb[1:] = np.cumsum(cap)
    L = int(tsb[-1])
    NB = L // 128

    # per-edge rank within (core, cell) -> stream slot
    sk2 = chw[order]
    g2_start = np.r_[0, np.flatnonzero(np.diff(sk2)) + 1]
    g2_id = np.cumsum(np.r_[0, np.diff(sk2) != 0])
    rank_in_cell = np.arange(EA) - g2_start[g2_id]
    slot = tsb[sk2 % NCELL] + rank_in_cell

    # fill per-core streams
    oc = core_e[order]
    o_ip, o_nrm, o_dl = ip[order], nrm[order], dl[order]
    ip_st = np.zeros((NCORE, L), np.int64)
    nrm_st = np.zeros((NCORE, L), np.float32)
    dl_st = np.zeros((NCORE, L), np.int64)
    for c in range(NCORE):
        m = oc == c
        s = slot[m]
        ip_st[c, s] = o_ip[m]
        nrm_st[c, s] = o_nrm[m]
        dl_st[c, s] = o_dl[m]

    # sort slots within each 128-block by source index (HBM locality)
    ipb = ip_st.reshape(NCORE, NB, 128)
    so = np.argsort(ipb, axis=2, kind="stable")
    ip_st = np.take_along_axis(ipb, so, 2).reshape(NCORE, L)
    nrm_st = np.take_along_axis(nrm_st.reshape(NCORE, NB, 128), so, 2).reshape(NCORE, L)
    dl_st = np.take_along_axis(dl_st.reshape(NCORE, NB, 128), so, 2).reshape(NCORE, L)

    # uniform block structure: d0 / width per block (union over cores)
    real = nrm_st > 0
    dmask = np.where(real, dl_st, 1 << 30).reshape(NCORE, NB, 128)
    d0 = dmask.min(2).min(0)
    dmask = np.where(real, dl_st, -1).reshape(NCORE, NB, 128)
    dend = dmask.max(2).max(0) + 1
    none = dend <= 0
    d0[none] = 0
    dend[none] = 1
    d0 = np.minimum(d0, dend - 1)
    cb = dend - d0                          # [NB] block col width
    Cu = int(cb.max())

    # packed one-hot weight table
    offP = np.zeros(NB + 1, np.int64)
    offP[1:] = np.cumsum(cb)
    P = int(((offP[-1] + 15) // 16) * 16)
    sl = np.arange(L)
    bidx = sl // 128
    prt = sl % 128
    wsP = np.zeros((NCORE, 128, P), np.float16)
    par = np.zeros((NCORE, 128, NB), np.float16)
    for c in range(NCORE):
        ok = nrm_st[c] > 0
        colP = offP[bidx] + dl_st[c] - d0[bidx]
        wsP[c][prt[ok], colP[ok]] = nrm_st[c][ok]
        par[c][prt, bidx] = 1.0 - (ip_st[c] & 1)
    parO = (1.0 - par).astype(np.float16)
    # pad slots: parity pair must sum to <=1 per slot; pad slots have
    # par=1, parO=0 via ip=0 -> fine (ws=0 kills them anyway).

    def wrap_idx(a):
        v = a.astype(np.int16).reshape(-1, 16).T      # [16, L/16]
        return np.tile(v, (8, 1))                     # [128, L/16]

    idx1 = np.stack([wrap_idx(ip_st[c]) for c in range(NCORE)])
    idx2 = np.stack([wrap_idx(ip_st[c] // 2) for c in range(NCORE)])

    # per-cell metadata for the emitter
    cells = []
    for h in range(2):
        for wdw in range(W):
            ci = h * W + wdw
            s0, s1 = int(tsb[ci]), int(tsb[ci + 1])
            cells.append(dict(h=h, w=wdw, s0=s0, s1=s1,
                              b0=s0 // 128, b1=s1 // 128))

    pad_ratio = L / max(1.0, EA / NCORE)
    return dict(
        cfg=cfg, NP=NP, CA=CA, CB=CB, NA=NA, NBrows=NB_, W=W, L=L, NB=NB,
        Cu=Cu, tsb=tsb, d0=d0, cb=cb, offP=offP, P=P, cells=cells,
        wsP=wsP, par=par, parO=parO, idx1=idx1, idx2=idx2,
        posfull=posfull, pad_ratio=float(pad_ratio),
    )


# ------------------------------------------------------------ bass program --
STAGES = ["conv1", "bn1", "ag", "conv2", "full"]


def build(st, stage="full", reps=1):
    slev = STAGES.index(stage)
    cfg = st["cfg"]
    N, F, H, O, NCORE = cfg["N"], cfg["F"], cfg["H"], cfg["O"], cfg["NCORE"]
    EPS, NEG, WIN, GCH = cfg["EPS"], cfg["NEG"], cfg["WIN"], cfg["GCH"]
    NP, CA, CB, NA = st["NP"], st["CA"], st["CB"], st["NA"]
    NBr = st["NBrows"]
    W, L, NB = st["W"], st["L"], st["NB"]
    d0s, cbs, offP, P = st["d0"], st["cb"], st["offP"], st["P"]
    cells = st["cells"]
    rg = [list(range(NCORE))]

    nc = bacc.Bacc("TRN2", target_bir_lowering=False, debug=False,
                   num_devices=NCORE, num_swdge_queues=4)
    NQ = 4
    qctr = [0]

    # --- I/O ---
    x16_d = nc.dram_tensor("x16", [N, F], F16, kind="ExternalInput")
    W1 = nc.dram_tensor("w1", [F, H], FP32, kind="ExternalInput")
    g1 = nc.dram_tensor("g1", [H], FP32, kind="ExternalInput")
    be1 = nc.dram_tensor("beta1", [H], FP32, kind="ExternalInput")
    W2 = nc.dram_tensor("w2m", [H, O], FP32, kind="ExternalInput")
    g2 = nc.dram_tensor("g2", [O], FP32, kind="ExternalInput")
    be2 = nc.dram_tensor("beta2", [O], FP32, kind="ExternalInput")
    i1_d = nc.dram_tensor("idx1", [128, L // 16], I16, kind="ExternalInput")
    i2_d = nc.dram_tensor("idx2", [128, L // 16], I16, kind="ExternalInput")
    wsP_d = nc.dram_tensor("wsP", [128, P], F16, kind="ExternalInput")
    par_d = nc.dram_tensor("par", [128, NB], F16, kind="ExternalInput")
    parO_d = nc.dram_tensor("parO", [128, NB], F16, kind="ExternalInput")
    out_d = nc.dram_tensor("out", [NP, O], FP32, kind="ExternalOutput")

    def bcast_inner(ap, k):
        return bass.AP(tensor=ap.tensor, offset=ap.offset, ap=ap.ap + [[0, k]])

    with tile.TileContext(nc) as tc:
        sing = tc.alloc_tile_pool(name="sing", bufs=1)
        small = tc.alloc_tile_pool(name="small", bufs=3)
        gbuf_p = tc.alloc_tile_pool(name="gbuf", bufs=10)
        gsel_p = tc.alloc_tile_pool(name="gsel", bufs=6)
        zpool = tc.alloc_tile_pool(name="zpool", bufs=1)
        sq_p = tc.alloc_tile_pool(name="sqp", bufs=2)
        ptile = tc.alloc_tile_pool(name="ptile", bufs=2, space="PSUM")
        pmisc = tc.alloc_tile_pool(name="pmisc", bufs=2, space="PSUM")
        pyc = tc.alloc_tile_pool(name="pyc", bufs=2, space="PSUM")
        dram = tc.alloc_tile_pool(name="dram", bufs=1, space="DRAM")

        # --- persistent DRAM scratch ---
        mo_in = dram.tile([128, 2], FP32)
        mo_out = dram.tile([128, 2], FP32)
        mo2_in = dram.tile([O, 2], FP32)
        mo2_out = dram.tile([O, 2], FP32)
        u2A_d = dram.tile([CA, O], F16)
        u2B_d = dram.tile([CB, O], F16)

        # --- constants ---
        from concourse.masks import make_identity
        ident = sing.tile([128, 128], FP32)
        make_identity(nc, ident[:])
        ident_h = sing.tile([128, 128], F16)
        nc.vector.tensor_copy(out=ident_h[:], in_=ident[:])
        eps_sb = sing.tile([128, 1], FP32)
        nc.vector.memset(eps_sb[:], EPS)

        W1_sb = sing.tile([F, H], FP32)
        nc.sync.dma_start(out=W1_sb[:], in_=W1[:, :])
        W1_16 = sing.tile([F, H], F16)
        nc.vector.tensor_copy(out=W1_16[:], in_=W1_sb[:])
        W2_sb = sing.tile([H, O], FP32)
        nc.sync.dma_start(out=W2_sb[:], in_=W2[:, :])
        W2_16 = sing.tile([H, O], F16)
        nc.vector.tensor_copy(out=W2_16[:], in_=W2_sb[:])
        g1_sb = sing.tile([H, 1], FP32)
        nc.sync.dma_start(out=g1_sb[:], in_=g1[:, None])
        be1_sb = sing.tile([H, 1], FP32)
        nc.sync.dma_start(out=be1_sb[:], in_=be1[:, None])
        g2_sb = sing.tile([O, 1], FP32)
        nc.sync.dma_start(out=g2_sb[:], in_=g2[:, None])
        be2_sb = sing.tile([O, 1], FP32)
        nc.sync.dma_start(out=be2_sb[:], in_=be2[:, None])

        # --- stream metadata ---
        i1_sb = sing.tile([128, L // 16], I16)
        nc.sync.dma_start(out=i1_sb[:], in_=i1_d[:, :])
        i2_sb = sing.tile([128, L // 16], I16)
        nc.sync.dma_start(out=i2_sb[:], in_=i2_d[:, :])
        wsP_sb = sing.tile([128, P], F16)
        nc.sync.dma_start(out=wsP_sb[:], in_=wsP_d[:, :])
        par_sb = sing.tile([128, NB], F16)
        nc.sync.dma_start(out=par_sb[:], in_=par_d[:, :])
        parO_sb = sing.tile([128, NB], F16)
        nc.sync.dma_start(out=parO_sb[:], in_=parO_d[:, :])

        def wlen(wdw):
            return min(WIN, NP - wdw * WIN)

        def emit_once():
            u2fA = dram.tile([NA, O], F16, addr_space="Shared", tag=None,
                             uniquify=True, name="u2fA")
            u2fB = dram.tile([NBr, O], F16, addr_space="Shared", tag=None,
                             uniquify=True, name="u2fB")

            # =============== conv aggregation (shared emitter) ==============
            def conv(layer, zT, HH, after_close=None):
                """Aggregate into zT ([HH, NP] slice of an [128, NP] f16
                tile) using the shared slot streams.  after_close(wdw) is
                invoked after each chunk-B window close (window final)."""
                for cell in cells:
                    h, wdw = cell["h"], cell["w"]
                    s0, s1, b0 = cell["s0"], cell["s1"], cell["b0"]
                    if layer == 1:
                        src_ap = (x16_d[0:NA, :] if h == 0
                                  else x16_d[NA:N, :])
                        idx_sb = i1_sb
                    else:
                        u2f = u2fA if h == 0 else u2fB
                        npair = (NA if h == 0 else NBr) // 2
                        u2ap = u2f[:]
                        src_ap = bass.AP(
                            tensor=u2ap.tensor, offset=u2ap.offset,
                            ap=[[128, npair], [1, 128]])
                        idx_sb = i2_sb
                    # gather calls + their block matmuls
                    pz = ptile.tile([128, WIN], FP32, tag="pz")
                    nmm = (s1 - s0) // 128
                    mi = 0
                    for o in range(s0, s1, GCH):
                        ni = min(GCH, s1 - o)
                        gb = gbuf_p.tile([128, GCH // 128, F], F16, tag="gb")
                        nc.gpsimd.dma_gather(
                            out_ap=gb[:, 0:ni // 128, :],
                            in_ap=src_ap,
                            idxs_ap=idx_sb[:, o // 16:(o + ni) // 16],
                            num_idxs=ni, num_idxs_reg=ni, elem_size=F,
                            single_packet=False,
                            queue_num=qctr[0] % NQ)
                        qctr[0] += 1
                        cb0 = o // 128          # absolute block index
                        nbk = ni // 128
                        if layer == 2:
                            gs = gsel_p.tile([128, GCH // 128, O], F16,
                                             tag="gs")
                            gt = gsel_p.tile([128, GCH // 128, O], F16,
                                             tag="gt")
                            nc.vector.tensor_tensor(
                                out=gs[:, 0:nbk, :],
                                in0=gb[:, 0:nbk, 0:O],
                                in1=bcast_inner(par_sb[:, cb0:cb0 + nbk], O),
                                op=mybir.AluOpType.mult)
                            nc.vector.tensor_tensor(
                                out=gt[:, 0:nbk, :],
                                in0=gb[:, 0:nbk, O:2 * O],
                                in1=bcast_inner(parO_sb[:, cb0:cb0 + nbk], O),
                                op=mybir.AluOpType.mult)
                            nc.vector.tensor_tensor(
                                out=gs[:, 0:nbk, :], in0=gs[:, 0:nbk, :],
                                in1=gt[:, 0:nbk, :],
                                op=mybir.AluOpType.add)
                        for j in range(nbk):
                            b = cb0 + j
                            dd0 = int(d0s[b])
                            cbw = int(cbs[b])
                            po = int(offP[b])
                            if layer == 1:
                                nc.tensor.matmul(
                                    pz[:, dd0:dd0 + cbw],
                                    lhsT=gb[:, j, :],
                                    rhs=wsP_sb[:, po:po + cbw],
                                    start=(mi == 0), stop=(mi == nmm - 1),
                                    skip_group_check=True)
                            else:
                                nc.tensor.matmul(
                                    pz[:O, dd0:dd0 + cbw],
                                    lhsT=gs[:, j, :],
                                    rhs=wsP_sb[:, po:po + cbw],
                                    start=(mi == 0), stop=(mi == nmm - 1),
                                    skip_group_check=True)
                            mi += 1
                    # close window: chunk A copies, chunk B adds
                    wl = wlen(wdw)
                    dst = zT[:HH, wdw * WIN:wdw * WIN + wl]
                    if h == 0:
                        nc.scalar.activation(
                            out=dst, in_=pz[:HH, :wl],
                            func=mybir.ActivationFunctionType.Identity)
                    else:
                        nc.vector.tensor_tensor(
                            out=dst, in0=pz[:HH, :wl], in1=dst,
                            op=mybir.AluOpType.add)
                        if after_close is not None:
                            after_close(wdw)

            def stat_window(acc, src, HH, wdw):
                """DVE sum / sum-sq of one window into acc columns."""
                wl = wlen(wdw)
                seg = src[:HH, wdw * WIN:wdw * WIN + wl]
                nc.vector.tensor_reduce(
                    out=acc[:HH, wdw:wdw + 1], in_=seg,
                    op=mybir.AluOpType.add, axis=mybir.AxisListType.XYZW)
                sq = sq_p.tile([128, WIN], FP32, tag="sq")
                nc.vector.tensor_tensor(out=sq[:HH, 0:wl], in0=seg,
                                        in1=seg, op=mybir.AluOpType.mult)
                nc.vector.tensor_reduce(
                    out=acc[:HH, W + wdw:W + wdw + 1], in_=sq[:HH, 0:wl],
                    op=mybir.AluOpType.add, axis=mybir.AxisListType.XYZW)

            def stats_ar(acc, HH, min_d, mout_d, tag):
                """Final reduce of per-window stats -> AllReduce'd [HH,2]."""
                s_sb = small.tile([128, 2], FP32, tag="stat")
                nc.vector.tensor_reduce(
                    out=s_sb[:HH, 0:1], in_=acc[:HH, 0:W],
                    op=mybir.AluOpType.add, axis=mybir.AxisListType.XYZW)
                nc.vector.tensor_reduce(
                    out=s_sb[:HH, 1:2], in_=acc[:HH, W:2 * W],
                    op=mybir.AluOpType.add, axis=mybir.AxisListType.XYZW)
                nc.sync.dma_start(out=min_d[:, :], in_=s_sb[:HH, :])
                nc.gpsimd.collective_compute(
                    "AllReduce", mybir.AluOpType.add, replica_groups=rg,
                    ins=[min_d.opt()], outs=[mout_d.opt()])
                mg = small.tile([128, 2], FP32, tag=tag)
                nc.sync.dma_start(out=mg[:HH, 0:2], in_=mout_d[:, :])
                return mg

            def bn_fold(mg, HH, g_sb, be_sb):
                """scale/shift from AllReduce'd [HH,2] raw moments."""
                mu = small.tile([128, 1], FP32, tag="mu")
                nc.vector.tensor_scalar_mul(out=mu[:HH], in0=mg[:HH, 0:1],
                                            scalar1=1.0 / N)
                var = small.tile([128, 1], FP32, tag="var")
                nc.vector.tensor_scalar_mul(out=var[:HH], in0=mg[:HH, 1:2],
                                            scalar1=1.0 / N)
                mu2 = small.tile([128, 1], FP32, tag="mu2")
                nc.vector.tensor_mul(mu2[:HH], mu[:HH], mu[:HH])
                nc.vector.tensor_sub(var[:HH], var[:HH], mu2[:HH])
                sqv = small.tile([128, 1], FP32, tag="sqv")
                nc.scalar.activation(out=sqv[:HH], in_=var[:HH],
                                     func=mybir.ActivationFunctionType.Sqrt,
                                     bias=eps_sb[:HH])
                s_sb = small.tile([128, 1], FP32, tag="s")
                nc.vector.reciprocal(out=s_sb[:HH], in_=sqv[:HH])
                nc.vector.tensor_mul(s_sb[:HH], s_sb[:HH], g_sb[:HH])
                tb_sb = small.tile([128, 1], FP32, tag="tb")
                nc.vector.tensor_mul(tb_sb[:HH], mu[:HH], s_sb[:HH])
                nc.vector.tensor_sub(tb_sb[:HH], be_sb[:HH], tb_sb[:HH])
                return s_sb, tb_sb

            # ---- layer 1 ----
            zT1 = zpool.tile([128, NP], F16, tag="zbig")
            y1sb = zpool.tile([128, NP], F16, tag="y1")
            acc1 = small.tile([128, 2 * W], FP32, tag="statw1")

            def close1(wdw):
                # y1 = W1^T z1 for this window + its stats (under DMA shadow)
                if slev < 1:
                    return
                wl = wlen(wdw)
                py = pyc.tile([128, WIN], FP32, tag="py")
                nc.tensor.matmul(py[:, 0:wl], lhsT=W1_16[:],
                                 rhs=zT1[:, wdw * WIN:wdw * WIN + wl],
                                 start=True, stop=True,
                                 skip_group_check=True)
                nc.vector.tensor_copy(out=y1sb[:, wdw * WIN:wdw * WIN + wl],
                                      in_=py[:, 0:wl])
                stat_window(acc1, y1sb, H, wdw)

            conv(1, zT1, H, after_close=close1)
            if slev >= 1:
                mg1 = stats_ar(acc1, H, mo_in, mo_out, "mg1")
                s1, tb1 = bn_fold(mg1, H, g1_sb, be1_sb)
                # u = LeakyReLU(BN(y1)); u2 = u @ W2; write chunk rows
                for wdw in range(W):
                    wl = wlen(wdw)
                    u = small.tile([128, WIN], F16, tag="u")
                    nc.scalar.activation(
                        out=u[:, 0:wl], in_=y1sb[:, wdw * WIN:wdw * WIN + wl],
                        func=mybir.ActivationFunctionType.Identity,
                        scale=s1[:H], bias=tb1[:H])
                    v = small.tile([128, WIN], F16, tag="v")
                    nc.vector.tensor_scalar_mul(out=v[:, 0:wl], in0=u[:, 0:wl],
                                                scalar1=NEG)
                    nc.vector.tensor_tensor(out=u[:, 0:wl], in0=u[:, 0:wl],
                                            in1=v[:, 0:wl],
                                            op=mybir.AluOpType.max)
                    p2 = pyc.tile([128, WIN], FP32, tag="py")
                    nc.tensor.matmul(p2[:O, 0:wl], lhsT=W2_16[:],
                                     rhs=u[:, 0:wl], start=True, stop=True,
                                     skip_group_check=True)
                    u2sb = small.tile([128, WIN], F16, tag="u2sb")
                    nc.vector.tensor_copy(out=u2sb[:O, 0:wl], in_=p2[:O, 0:wl])
                    for tb_ in range(0, wl, 128):
                        n0 = wdw * WIN + tb_
                        tn = min(128, NP - n0)
                        po = pmisc.tile([128, 128], F16, tag="ptr")
                        nc.tensor.transpose(po[:tn, :O],
                                            u2sb[:O, tb_:tb_ + tn],
                                            ident_h[:O, :O])
                        xo = small.tile([128, O], F16, tag="xo")
                        nc.vector.tensor_copy(out=xo[:tn], in_=po[:tn, :O])
                        if n0 < CA:
                            nc.sync.dma_start(out=u2A_d[n0:n0 + tn, :],
                                              in_=xo[:tn])
                        else:
                            nc.sync.dma_start(out=u2B_d[n0 - CA:n0 - CA + tn, :],
                                              in_=xo[:tn])
            if slev >= 2:
                nc.gpsimd.collective_compute(
                    "AllGather", mybir.AluOpType.bypass, replica_groups=rg,
                    ins=[u2A_d.opt()], outs=[u2fA.opt()])
                nc.gpsimd.collective_compute(
                    "AllGather", mybir.AluOpType.bypass, replica_groups=rg,
                    ins=[u2B_d.opt()], outs=[u2fB.opt()])

            # ---- layer 2 ----
            if slev >= 3:
                zT2 = zpool.tile([128, NP], F16, tag="z2")
                acc2 = small.tile([128, 2 * W], FP32, tag="statw2")

                def close2(wdw):
                    if slev >= 4:
                        stat_window(acc2, zT2, O, wdw)

                conv(2, zT2, O, after_close=close2)
            if slev >= 4:
                mg2 = stats_ar(acc2, O, mo2_in, mo2_out, "mg2")
                s2, tb2 = bn_fold(mg2, O, g2_sb, be2_sb)
                y2 = zpool.tile([128, NP], F16, tag="y2o")
                nc.scalar.activation(out=y2[:O, :], in_=zT2[:O, :],
                                     func=mybir.ActivationFunctionType.Identity,
                                     scale=s2[:O], bias=tb2[:O])
                for tb_ in range(0, NP, 128):
                    tn = min(128, NP - tb_)
                    po = pmisc.tile([128, 128], F16, tag="ptr")
                    nc.tensor.transpose(po[:tn, :O], y2[:O, tb_:tb_ + tn],
                                        ident_h[:O, :O])
                    oo = small.tile([128, O], FP32, tag="oo")
                    nc.vector.tensor_copy(out=oo[:tn], in_=po[:tn, :O])
                    nc.sync.dma_start(out=out_d[tb_:tb_ + tn, :], in_=oo[:tn])

        for _rep in range(reps):
            emit_once()

        for p in (dram, pyc, pmisc, ptile, sq_p, zpool, gsel_p, gbuf_p,
                  small, sing):
            p.release()

    nc.compile()
    return nc


# ------------------------------------------------------------------ runner --
def make_in_maps(st, inputs):
    cfg = st["cfg"]
    NCORE = cfg["NCORE"]
    x16n = np.asarray(inputs["drug_smiles_fea"], np.float32).astype(np.float16)
    x16 = np.empty_like(x16n)
    x16[st["posfull"]] = x16n                 # chunk-permuted rows
    maps = []
    for c in range(NCORE):
        maps.append(dict(
            x16=x16,
            w1=np.asarray(inputs["W1"], np.float32),
            g1=np.asarray(inputs["g1"], np.float32),
            beta1=np.asarray(inputs["beta1"], np.float32),
            w2m=np.asarray(inputs["W2"], np.float32),
            g2=np.asarray(inputs["g2"], np.float32),
            beta2=np.asarray(inputs["beta2"], np.float32),
            idx1=st["idx1"][c], idx2=st["idx2"][c],
            wsP=np.ascontiguousarray(st["wsP"][c]),
            par=np.ascontiguousarray(st["par"][c]),
            parO=np.ascontiguousarray(st["parO"][c]),
        ))
    return maps


_LAST = {}


def kernel(**inputs):
    cfg = CFG
    adj = np.asarray(inputs["ATC_adj"])
    w = np.asarray(inputs["ATC_weight"], np.float32)
    st = preprocess(adj, w, cfg)
    nc = build(st)
    maps = make_in_maps(st, inputs)
    res = bass_utils.run_bass_kernel_spmd(
        nc, maps, core_ids=list(range(cfg["NCORE"])))
    out = np.concatenate([res.results[c]["out"] for c in range(cfg["NCORE"])], 0)
    _LAST.update(st=st, nc=nc, maps=maps)
    return out


# revision 17
# speedup vs baseline: 1.0253x; 1.0253x over previous
"""Trainium2 Bass kernel for nn_ATC_Network (2-layer GCN + BN + LeakyReLU).

Computation (see reference):
    row, col, w  (+ self loops w=1)
    deg[c] = sum_{e: col=c} w_e ;  dis = rsqrt(deg)
    norm_e = dis[row]*w*dis[col]
    z1[c]  = sum_e norm_e * x[row]          (conv1 aggregate, incl self loop)
    y1     = z1 @ W1 ; x2 = LeakyReLU(BN(y1))        (conv bias cancels in BN)
    z2[c]  = sum_e norm_e * (x2 @ W2)[row]
    out    = BN(z2)

v3 design notes:
  - dst-sharded across 8 cores (NP = 6250 dst nodes per core); graph norm
    host-side; self-loops are ordinary edges.
  - ONE padded slot-stream per core drives BOTH conv layers (shared
    weighted one-hot table ws).  Cells are (src-chunk h, dst-window w):
    13 windows of 512 dsts, 2 src chunks.  Slots within a 128-block are
    sorted by source index for HBM locality.
  - conv1 gathers 256B x16 rows from a chunk-permuted x16_arr (chunk-local
    indices fit int16).  PSUM [128,512] per window; chunk-A pass closes
    with a copy, chunk-B pass closes with an add.
  - conv2 pre-folds W2: u2 = x2@W2 (64 wide), AllGathered f16 in TWO
    chunks (A = local rows [0:3072] = windows 0-5) so the second AG
    overlaps conv2's chunk-A gathers.  Gathers fetch 256B pairs of u2
    rows; a DVE parity select picks the right 64 columns.
  - BN stats: DVE free-dim reduces over the feature-major activations;
    AllReduce payload is [128,2] f32.
"""

import sys

sys.path.insert(0, "/opt/trn_rl_repo")

import numpy as np
import ml_dtypes

import concourse.bass as bass
import concourse.tile as tile
from concourse import bacc, bass_utils, mybir

FP32 = mybir.dt.float32
F16 = mybir.dt.float16
I16 = mybir.dt.int16

# ---------------------------------------------------------------- config ----
CFG = dict(
    N=50000, E=800000, F=128, H=128, O=64, NCORE=8,
    WIN=512,             # dsts per PSUM window
    CA=3072,             # chunk-A rows per core (= 6 windows)
    GCH=1024,            # slots per dma_gather call (SWDGE ring: 1024 descs)
    EPS=1e-5, NEG=0.01,
)


# ---------------------------------------------------------- preprocessing ---
def preprocess(adj, w, cfg):
    """Host-side graph prep: norm, self-loops, per-core padded slot streams.

    Uniform block structure across cores (SPMD: one instruction stream;
    per-core data differs).
    """
    N, E, NCORE = cfg["N"], cfg["E"], cfg["NCORE"]
    WIN, CA = cfg["WIN"], cfg["CA"]
    NP = N // NCORE
    CB = NP - CA
    NA, NB_ = NCORE * CA, NCORE * CB        # chunk region sizes (24576, 25424)
    W = (NP + WIN - 1) // WIN               # 13 windows
    NCELL = 2 * W

    row0 = np.asarray(adj[0], np.int64)
    col0 = np.asarray(adj[1], np.int64)
    w = np.asarray(w, np.float32)

    # gcn_norm host-side (incl self loops, weight 1)
    deg = np.zeros(N, np.float64)
    np.add.at(deg, col0, w.astype(np.float64))
    deg += 1.0
    dis = (1.0 / np.sqrt(deg)).astype(np.float32)

    row = np.concatenate([row0, np.arange(N, dtype=np.int64)])
    col = np.concatenate([col0, np.arange(N, dtype=np.int64)])
    nrm = np.concatenate([dis[row0] * w * dis[col0], dis * dis]).astype(np.float32)
    EA = E + N

    # chunk-permuted source positions
    g = np.arange(N, dtype=np.int64)
    core_g, loc_g = g // NP, g % NP
    inA = loc_g < CA
    posA = core_g * CA + loc_g
    posB = core_g * CB + (loc_g - CA)
    poschunk = np.where(inA, posA, posB)    # position within own chunk region
    posfull = np.where(inA, posA, NA + posB)  # row in x16_arr

    # per-edge attributes
    core_e = col // NP
    lc = col % NP
    w_e = lc // WIN
    dl = lc - w_e * WIN
    h_e = (~inA[row]).astype(np.int64)      # src chunk
    ip = poschunk[row]                      # chunk-local gather index (conv1)

    # stable sort by (core, h, w, dl)
    key = ((core_e * 2 + h_e) * W + w_e) * WIN + dl
    order = np.argsort(key, kind="stable")

    # caps per cell (h, w): max over cores, ceil to 128
    chw = (core_e * 2 + h_e) * W + w_e
    cnt = np.bincount(chw, minlength=NCORE * NCELL).reshape(NCORE, NCELL)
    cap = np.maximum(cnt.max(0), 1)
    cap = ((cap + 127) // 128) * 128        # [NCELL]
    tsb = np.zeros(NCELL + 1, np.int64)
    tsb[1:] = np.cumsum(cap)
    L = int(tsb[-1])
    NB = L // 128

    # per-edge rank within (core, cell) -> stream slot
    sk2 = chw[order]
    g2_start = np.r_[0, np.flatnonzero(np.diff(sk2)) + 1]
    g2_id = np.cumsum(np.r_[0, np.diff(sk2) != 0])
    rank_in_cell = np.arange(EA) - g2_start[g2_id]
    slot = tsb[sk2 % NCELL] + rank_in_cell

    # fill per-core streams
    oc = core_e[order]
    o_ip, o_nrm, o_dl = ip[order], nrm[order], dl[order]
    ip_st = np.zeros((NCORE, L), np.int64)
    nrm_st = np.zeros((NCORE, L), np.float32)
    dl_st = np.zeros((NCORE, L), np.int64)
    for c in range(NCORE):
        m = oc == c
        s = slot[m]
        ip_st[c, s] = o_ip[m]
        nrm_st[c, s] = o_nrm[m]
        dl_st[c, s] = o_dl[m]

    # sort slots within each 128-block by source index (HBM locality)
    ipb = ip_st.reshape(NCORE, NB, 128)
    so = np.argsort(ipb, axis=2, kind="stable")
    ip_st = np.take_along_axis(ipb, so, 2).reshape(NCORE, L)
    nrm_st = np.take_along_axis(nrm_st.reshape(NCORE, NB, 128), so, 2).reshape(NCORE, L)
    dl_st = np.take_along_axis(dl_st.reshape(NCORE, NB, 128), so, 2).reshape(NCORE, L)

    # uniform block structure: d0 / width per block (union over cores)
    real = nrm_st > 0
    dmask = np.where(real, dl_st, 1 << 30).reshape(NCORE, NB, 128)
    d0 = dmask.min(2).min(0)
    dmask = np.where(real, dl_st, -1).reshape(NCORE, NB, 128)
    dend = dmask.max(2).max(0) + 1
    none = dend <= 0
    d0[none] = 0
    dend[none] = 1
    d0 = np.minimum(d0, dend - 1)
    cb = dend - d0                          # [NB] block col width
    Cu = int(cb.max())

    # packed one-hot weight table
    offP = np.zeros(NB + 1, np.int64)
    offP[1:] = np.cumsum(cb)
    P = int(((offP[-1] + 15) // 16) * 16)
    sl = np.arange(L)
    bidx = sl // 128
    prt = sl % 128
    wsP = np.zeros((NCORE, 128, P), np.float16)
    par = np.zeros((NCORE, 128, NB), np.float16)
    for c in range(NCORE):
        ok = nrm_st[c] > 0
        colP = offP[bidx] + dl_st[c] - d0[bidx]
        wsP[c][prt[ok], colP[ok]] = nrm_st[c][ok]
        par[c][prt, bidx] = 1.0 - (ip_st[c] & 1)
    parO = (1.0 - par).astype(np.float16)
    # pad slots: parity pair must sum to <=1 per slot; pad slots have
    # par=1, parO=0 via ip=0 -> fine (ws=0 kills them anyway).

    def wrap_idx(a):
        v = a.astype(np.int16).reshape(-1, 16).T      # [16, L/16]
        return np.tile(v, (8, 1))                     # [128, L/16]

    idx1 = np.stack([wrap_idx(ip_st[c]) for c in range(NCORE)])
    idx2 = np.stack([wrap_idx(ip_st[c] // 2) for c in range(NCORE)])

    # per-cell metadata for the emitter
    cells = []
    for h in range(2):
        for wdw in range(W):
            ci = h * W + wdw
            s0, s1 = int(tsb[ci]), int(tsb[ci + 1])
            cells.append(dict(h=h, w=wdw, s0=s0, s1=s1,
                              b0=s0 // 128, b1=s1 // 128))

    pad_ratio = L / max(1.0, EA / NCORE)
    return dict(
        cfg=cfg, NP=NP, CA=CA, CB=CB, NA=NA, NBrows=NB_, W=W, L=L, NB=NB,
        Cu=Cu, tsb=tsb, d0=d0, cb=cb, offP=offP, P=P, cells=cells,
        wsP=wsP, par=par, parO=parO, idx1=idx1, idx2=idx2,
        posfull=posfull, pad_ratio=float(pad_ratio),
    )


# ------------------------------------------------------------ bass program --
STAGES = ["conv1", "bn1", "ag", "conv2", "full"]


def build(st, stage="full", reps=1):
    slev = STAGES.index(stage)
    cfg = st["cfg"]
    N, F, H, O, NCORE = cfg["N"], cfg["F"], cfg["H"], cfg["O"], cfg["NCORE"]
    EPS, NEG, WIN, GCH = cfg["EPS"], cfg["NEG"], cfg["WIN"], cfg["GCH"]
    NP, CA, CB, NA = st["NP"], st["CA"], st["CB"], st["NA"]
    NBr = st["NBrows"]
    W, L, NB = st["W"], st["L"], st["NB"]
    d0s, cbs, offP, P = st["d0"], st["cb"], st["offP"], st["P"]
    cells = st["cells"]
    rg = [list(range(NCORE))]

    nc = bacc.Bacc("TRN2", target_bir_lowering=False, debug=False,
                   num_devices=NCORE, num_swdge_queues=4)
    NQ = 4
    qctr = [0]

    # --- I/O ---
    x16_d = nc.dram_tensor("x16", [N, F], F16, kind="ExternalInput")
    W1 = nc.dram_tensor("w1", [F, H], FP32, kind="ExternalInput")
    g1 = nc.dram_tensor("g1", [H], FP32, kind="ExternalInput")
    be1 = nc.dram_tensor("beta1", [H], FP32, kind="ExternalInput")
    W2 = nc.dram_tensor("w2m", [H, O], FP32, kind="ExternalInput")
    g2 = nc.dram_tensor("g2", [O], FP32, kind="ExternalInput")
    be2 = nc.dram_tensor("beta2", [O], FP32, kind="ExternalInput")
    i1_d = nc.dram_tensor("idx1", [128, L // 16], I16, kind="ExternalInput")
    i2_d = nc.dram_tensor("idx2", [128, L // 16], I16, kind="ExternalInput")
    wsP_d = nc.dram_tensor("wsP", [128, P], F16, kind="ExternalInput")
    par_d = nc.dram_tensor("par", [128, NB], F16, kind="ExternalInput")
    parO_d = nc.dram_tensor("parO", [128, NB], F16, kind="ExternalInput")
    out_d = nc.dram_tensor("out", [NP, O], FP32, kind="ExternalOutput")

    def bcast_inner(ap, k):
        return bass.AP(tensor=ap.tensor, offset=ap.offset, ap=ap.ap + [[0, k]])

    with tile.TileContext(nc) as tc:
        sing = tc.alloc_tile_pool(name="sing", bufs=1)
        small = tc.alloc_tile_pool(name="small", bufs=3)
        gbuf_p = tc.alloc_tile_pool(name="gbuf", bufs=16)
        gsel_p = tc.alloc_tile_pool(name="gsel", bufs=8)
        zpool = tc.alloc_tile_pool(name="zpool", bufs=1)
        sq_p = tc.alloc_tile_pool(name="sqp", bufs=2)
        ptile = tc.alloc_tile_pool(name="ptile", bufs=2, space="PSUM")
        pmisc = tc.alloc_tile_pool(name="pmisc", bufs=2, space="PSUM")
        pyc = tc.alloc_tile_pool(name="pyc", bufs=2, space="PSUM")
        dram = tc.alloc_tile_pool(name="dram", bufs=1, space="DRAM")

        # --- persistent DRAM scratch ---
        mo_in = dram.tile([128, 2], FP32)
        mo_out = dram.tile([128, 2], FP32)
        mo2_in = dram.tile([O, 2], FP32)
        mo2_out = dram.tile([O, 2], FP32)
        u2A_d = dram.tile([CA, O], F16)
        u2B_d = dram.tile([CB, O], F16)

        # --- constants ---
        from concourse.masks import make_identity
        ident = sing.tile([128, 128], FP32)
        make_identity(nc, ident[:])
        ident_h = sing.tile([128, 128], F16)
        nc.vector.tensor_copy(out=ident_h[:], in_=ident[:])
        eps_sb = sing.tile([128, 1], FP32)
        nc.vector.memset(eps_sb[:], EPS)

        W1_sb = sing.tile([F, H], FP32)
        nc.sync.dma_start(out=W1_sb[:], in_=W1[:, :])
        W1_16 = sing.tile([F, H], F16)
        nc.vector.tensor_copy(out=W1_16[:], in_=W1_sb[:])
        W2_sb = sing.tile([H, O], FP32)
        nc.sync.dma_start(out=W2_sb[:], in_=W2[:, :])
        W2_16 = sing.tile([H, O], F16)
        nc.vector.tensor_copy(out=W2_16[:], in_=W2_sb[:])
        g1_sb = sing.tile([H, 1], FP32)
        nc.sync.dma_start(out=g1_sb[:], in_=g1[:, None])
        be1_sb = sing.tile([H, 1], FP32)
        nc.sync.dma_start(out=be1_sb[:], in_=be1[:, None])
        g2_sb = sing.tile([O, 1], FP32)
        nc.sync.dma_start(out=g2_sb[:], in_=g2[:, None])
        be2_sb = sing.tile([O, 1], FP32)
        nc.sync.dma_start(out=be2_sb[:], in_=be2[:, None])

        # --- stream metadata ---
        i1_sb = sing.tile([128, L // 16], I16)
        nc.sync.dma_start(out=i1_sb[:], in_=i1_d[:, :])
        i2_sb = sing.tile([128, L // 16], I16)
        nc.sync.dma_start(out=i2_sb[:], in_=i2_d[:, :])
        wsP_sb = sing.tile([128, P], F16)
        nc.sync.dma_start(out=wsP_sb[:], in_=wsP_d[:, :])
        par_sb = sing.tile([128, NB], F16)
        nc.sync.dma_start(out=par_sb[:], in_=par_d[:, :])
        parO_sb = sing.tile([128, NB], F16)
        nc.sync.dma_start(out=parO_sb[:], in_=parO_d[:, :])

        def wlen(wdw):
            return min(WIN, NP - wdw * WIN)

        def emit_once():
            u2fA = dram.tile([NA, O], F16, addr_space="Shared", tag=None,
                             uniquify=True, name="u2fA")
            u2fB = dram.tile([NBr, O], F16, addr_space="Shared", tag=None,
                             uniquify=True, name="u2fB")

            # =============== conv aggregation (shared emitter) ==============
            def conv(layer, zT, HH, after_close=None):
                """Aggregate into zT ([HH, NP] slice of an [128, NP] f16
                tile) using the shared slot streams.  after_close(wdw) is
                invoked after each chunk-B window close (window final)."""
                for cell in cells:
                    h, wdw = cell["h"], cell["w"]
                    s0, s1, b0 = cell["s0"], cell["s1"], cell["b0"]
                    if layer == 1:
                        src_ap = (x16_d[0:NA, :] if h == 0
                                  else x16_d[NA:N, :])
                        idx_sb = i1_sb
                    else:
                        u2f = u2fA if h == 0 else u2fB
                        npair = (NA if h == 0 else NBr) // 2
                        u2ap = u2f[:]
                        src_ap = bass.AP(
                            tensor=u2ap.tensor, offset=u2ap.offset,
                            ap=[[128, npair], [1, 128]])
                        idx_sb = i2_sb
                    # gather calls + their block matmuls
                    pz = ptile.tile([128, WIN], FP32, tag="pz")
                    nmm = (s1 - s0) // 128
                    mi = 0
                    for o in range(s0, s1, GCH):
                        ni = min(GCH, s1 - o)
                        gb = gbuf_p.tile([128, GCH // 128, F], F16, tag="gb")
                        nc.gpsimd.dma_gather(
                            out_ap=gb[:, 0:ni // 128, :],
                            in_ap=src_ap,
                            idxs_ap=idx_sb[:, o // 16:(o + ni) // 16],
                            num_idxs=ni, num_idxs_reg=ni, elem_size=F,
                            single_packet=False,
                            queue_num=qctr[0] % NQ)
                        qctr[0] += 1
                        cb0 = o // 128          # absolute block index
                        nbk = ni // 128
                        if layer == 2:
                            gs = gsel_p.tile([128, GCH // 128, O], F16,
                                             tag="gs")
                            gt = gsel_p.tile([128, GCH // 128, O], F16,
                                             tag="gt")
                            nc.vector.tensor_tensor(
                                out=gs[:, 0:nbk, :],
                                in0=gb[:, 0:nbk, 0:O],
                                in1=bcast_inner(par_sb[:, cb0:cb0 + nbk], O),
                                op=mybir.AluOpType.mult)
                            nc.vector.tensor_tensor(
                                out=gt[:, 0:nbk, :],
                                in0=gb[:, 0:nbk, O:2 * O],
                                in1=bcast_inner(parO_sb[:, cb0:cb0 + nbk], O),
                                op=mybir.AluOpType.mult)
                            nc.vector.tensor_tensor(
                                out=gs[:, 0:nbk, :], in0=gs[:, 0:nbk, :],
                                in1=gt[:, 0:nbk, :],
                                op=mybir.AluOpType.add)
                        for j in range(nbk):
                            b = cb0 + j
                            dd0 = int(d0s[b])
                            cbw = int(cbs[b])
                            po = int(offP[b])
                            if layer == 1:
                                nc.tensor.matmul(
                                    pz[:, dd0:dd0 + cbw],
                                    lhsT=gb[:, j, :],
                                    rhs=wsP_sb[:, po:po + cbw],
                                    start=(mi == 0), stop=(mi == nmm - 1),
                                    skip_group_check=True)
                            else:
                                nc.tensor.matmul(
                                    pz[:O, dd0:dd0 + cbw],
                                    lhsT=gs[:, j, :],
                                    rhs=wsP_sb[:, po:po + cbw],
                                    start=(mi == 0), stop=(mi == nmm - 1),
                                    skip_group_check=True)
                            mi += 1
                    # close window: chunk A copies, chunk B adds
                    wl = wlen(wdw)
                    dst = zT[:HH, wdw * WIN:wdw * WIN + wl]
                    if h == 0:
                        nc.scalar.activation(
                            out=dst, in_=pz[:HH, :wl],
                            func=mybir.ActivationFunctionType.Identity)
                    else:
                        nc.vector.tensor_tensor(
                            out=dst, in0=pz[:HH, :wl], in1=dst,
                            op=mybir.AluOpType.add)
                        if after_close is not None:
                            after_close(wdw)

            def stat_window(acc, src, HH, wdw):
                """DVE sum / sum-sq of one window into acc columns."""
                wl = wlen(wdw)
                seg = src[:HH, wdw * WIN:wdw * WIN + wl]
                nc.vector.tensor_reduce(
                    out=acc[:HH, wdw:wdw + 1], in_=seg,
                    op=mybir.AluOpType.add, axis=mybir.AxisListType.XYZW)
                sq = sq_p.tile([128, WIN], FP32, tag="sq")
                nc.vector.tensor_tensor(out=sq[:HH, 0:wl], in0=seg,
                                        in1=seg, op=mybir.AluOpType.mult)
                nc.vector.tensor_reduce(
                    out=acc[:HH, W + wdw:W + wdw + 1], in_=sq[:HH, 0:wl],
                    op=mybir.AluOpType.add, axis=mybir.AxisListType.XYZW)

            def stats_ar(acc, HH, min_d, mout_d, tag):
                """Final reduce of per-window stats -> AllReduce'd [HH,2]."""
                s_sb = small.tile([128, 2], FP32, tag="stat")
                nc.vector.tensor_reduce(
                    out=s_sb[:HH, 0:1], in_=acc[:HH, 0:W],
                    op=mybir.AluOpType.add, axis=mybir.AxisListType.XYZW)
                nc.vector.tensor_reduce(
                    out=s_sb[:HH, 1:2], in_=acc[:HH, W:2 * W],
                    op=mybir.AluOpType.add, axis=mybir.AxisListType.XYZW)
                nc.sync.dma_start(out=min_d[:, :], in_=s_sb[:HH, :])
                nc.gpsimd.collective_compute(
                    "AllReduce", mybir.AluOpType.add, replica_groups=rg,
                    ins=[min_d.opt()], outs=[mout_d.opt()])
                mg = small.tile([128, 2], FP32, tag=tag)
                nc.sync.dma_start(out=mg[:HH, 0:2], in_=mout_d[:, :])
                return mg

            def bn_fold(mg, HH, g_sb, be_sb):
                """scale/shift from AllReduce'd [HH,2] raw moments."""
                mu = small.tile([128, 1], FP32, tag="mu")
                nc.vector.tensor_scalar_mul(out=mu[:HH], in0=mg[:HH, 0:1],
                                            scalar1=1.0 / N)
                var = small.tile([128, 1], FP32, tag="var")
                nc.vector.tensor_scalar_mul(out=var[:HH], in0=mg[:HH, 1:2],
                                            scalar1=1.0 / N)
                mu2 = small.tile([128, 1], FP32, tag="mu2")
                nc.vector.tensor_mul(mu2[:HH], mu[:HH], mu[:HH])
                nc.vector.tensor_sub(var[:HH], var[:HH], mu2[:HH])
                sqv = small.tile([128, 1], FP32, tag="sqv")
                nc.scalar.activation(out=sqv[:HH], in_=var[:HH],
                                     func=mybir.ActivationFunctionType.Sqrt,
                                     bias=eps_sb[:HH])
                s_sb = small.tile([128, 1], FP32, tag="s")
                nc.vector.reciprocal(out=s_sb[:HH], in_=sqv[:HH])
                nc.vector.tensor_mul(s_sb[:HH], s_sb[:HH], g_sb[:HH])
                tb_sb = small.tile([128, 1], FP32, tag="tb")
                nc.vector.tensor_mul(tb_sb[:HH], mu[:HH], s_sb[:HH])
                nc.vector.tensor_sub(tb_sb[:HH], be_sb[:HH], tb_sb[:HH])
                return s_sb, tb_sb

            # ---- layer 1 ----
            zT1 = zpool.tile([128, NP], F16, tag="zbig")
            y1sb = zpool.tile([128, NP], F16, tag="y1")
            acc1 = small.tile([128, 2 * W], FP32, tag="statw1")

            def close1(wdw):
                # y1 = W1^T z1 for this window + its stats (under DMA shadow)
                if slev < 1:
                    return
                wl = wlen(wdw)
                py = pyc.tile([128, WIN], FP32, tag="py")
                nc.tensor.matmul(py[:, 0:wl], lhsT=W1_16[:],
                                 rhs=zT1[:, wdw * WIN:wdw * WIN + wl],
                                 start=True, stop=True,
                                 skip_group_check=True)
                nc.vector.tensor_copy(out=y1sb[:, wdw * WIN:wdw * WIN + wl],
                                      in_=py[:, 0:wl])
                stat_window(acc1, y1sb, H, wdw)

            conv(1, zT1, H, after_close=close1)
            if slev >= 1:
                mg1 = stats_ar(acc1, H, mo_in, mo_out, "mg1")
                s1, tb1 = bn_fold(mg1, H, g1_sb, be1_sb)
                # u = LeakyReLU(BN(y1)); u2 = u @ W2; write chunk rows
                for wdw in range(W):
                    wl = wlen(wdw)
                    u = small.tile([128, WIN], F16, tag="u")
                    nc.scalar.activation(
                        out=u[:, 0:wl], in_=y1sb[:, wdw * WIN:wdw * WIN + wl],
                        func=mybir.ActivationFunctionType.Identity,
                        scale=s1[:H], bias=tb1[:H])
                    v = small.tile([128, WIN], F16, tag="v")
                    nc.vector.tensor_scalar_mul(out=v[:, 0:wl], in0=u[:, 0:wl],
                                                scalar1=NEG)
                    nc.vector.tensor_tensor(out=u[:, 0:wl], in0=u[:, 0:wl],
                                            in1=v[:, 0:wl],
                                            op=mybir.AluOpType.max)
                    p2 = pyc.tile([128, WIN], FP32, tag="py")
                    nc.tensor.matmul(p2[:O, 0:wl], lhsT=W2_16[:],
                                     rhs=u[:, 0:wl], start=True, stop=True,
                                     skip_group_check=True)
                    u2sb = small.tile([128, WIN], F16, tag="u2sb")
                    nc.vector.tensor_copy(out=u2sb[:O, 0:wl], in_=p2[:O, 0:wl])
                    for tb_ in range(0, wl, 128):
                        n0 = wdw * WIN + tb_
                        tn = min(128, NP - n0)
                        po = pmisc.tile([128, 128], F16, tag="ptr")
                        nc.tensor.transpose(po[:tn, :O],
                                            u2sb[:O, tb_:tb_ + tn],
                                            ident_h[:O, :O])
                        xo = small.tile([128, O], F16, tag="xo")
                        nc.vector.tensor_copy(out=xo[:tn], in_=po[:tn, :O])
                        if n0 < CA:
                            nc.sync.dma_start(out=u2A_d[n0:n0 + tn, :],
                                              in_=xo[:tn])
                        else:
                            nc.sync.dma_start(out=u2B_d[n0 - CA:n0 - CA + tn, :],
                                              in_=xo[:tn])
            if slev >= 2:
                nc.gpsimd.collective_compute(
                    "AllGather", mybir.AluOpType.bypass, replica_groups=rg,
                    ins=[u2A_d.opt()], outs=[u2fA.opt()])
                nc.gpsimd.collective_compute(
                    "AllGather", mybir.AluOpType.bypass, replica_groups=rg,
                    ins=[u2B_d.opt()], outs=[u2fB.opt()])

            # ---- layer 2 ----
            if slev >= 3:
                zT2 = zpool.tile([128, NP], F16, tag="z2")
                acc2 = small.tile([128, 2 * W], FP32, tag="statw2")

                def close2(wdw):
                    if slev >= 4:
                        stat_window(acc2, zT2, O, wdw)

                conv(2, zT2, O, after_close=close2)
            if slev >= 4:
                mg2 = stats_ar(acc2, O, mo2_in, mo2_out, "mg2")
                s2, tb2 = bn_fold(mg2, O, g2_sb, be2_sb)
                y2 = zpool.tile([128, NP], F16, tag="y2o")
                nc.scalar.activation(out=y2[:O, :], in_=zT2[:O, :],
                                     func=mybir.ActivationFunctionType.Identity,
                                     scale=s2[:O], bias=tb2[:O])
                for tb_ in range(0, NP, 128):
                    tn = min(128, NP - tb_)
                    po = pmisc.tile([128, 128], F16, tag="ptr")
                    nc.tensor.transpose(po[:tn, :O], y2[:O, tb_:tb_ + tn],
                                        ident_h[:O, :O])
                    oo = small.tile([128, O], FP32, tag="oo")
                    nc.vector.tensor_copy(out=oo[:tn], in_=po[:tn, :O])
                    nc.sync.dma_start(out=out_d[tb_:tb_ + tn, :], in_=oo[:tn])

        for _rep in range(reps):
            emit_once()

        for p in (dram, pyc, pmisc, ptile, sq_p, zpool, gsel_p, gbuf_p,
                  small, sing):
            p.release()

    nc.compile()
    return nc


# ------------------------------------------------------------------ runner --
def make_in_maps(st, inputs):
    cfg = st["cfg"]
    NCORE = cfg["NCORE"]
    x16n = np.asarray(inputs["drug_smiles_fea"], np.float32).astype(np.float16)
    x16 = np.empty_like(x16n)
    x16[st["posfull"]] = x16n                 # chunk-permuted rows
    maps = []
    for c in range(NCORE):
        maps.append(dict(
            x16=x16,
            w1=np.asarray(inputs["W1"], np.float32),
            g1=np.asarray(inputs["g1"], np.float32),
            beta1=np.asarray(inputs["beta1"], np.float32),
            w2m=np.asarray(inputs["W2"], np.float32),
            g2=np.asarray(inputs["g2"], np.float32),
            beta2=np.asarray(inputs["beta2"], np.float32),
            idx1=st["idx1"][c], idx2=st["idx2"][c],
            wsP=np.ascontiguousarray(st["wsP"][c]),
            par=np.ascontiguousarray(st["par"][c]),
            parO=np.ascontiguousarray(st["parO"][c]),
        ))
    return maps


_LAST = {}


def kernel(**inputs):
    cfg = CFG
    adj = np.asarray(inputs["ATC_adj"])
    w = np.asarray(inputs["ATC_weight"], np.float32)
    st = preprocess(adj, w, cfg)
    nc = build(st)
    maps = make_in_maps(st, inputs)
    res = bass_utils.run_bass_kernel_spmd(
        nc, maps, core_ids=list(range(cfg["NCORE"])))
    out = np.concatenate([res.results[c]["out"] for c in range(cfg["NCORE"])], 0)
    _LAST.update(st=st, nc=nc, maps=maps)
    return out
